# revision 3
# baseline (speedup 1.0000x reference)
"""ContractiveREN Trainium2 kernel.

Host: derive REN model matrices from (X, Y, ...) in numpy (pre-scale by
1/Lambda, fold E_inv into the state-update matrices), transpose u to
[dims x time*batch] bf16, and ship ONLY the raw 32-row u to each core
(2 MB/core) -- the axon tunnel (~45 MB/s) is the bottleneck, so every
projection that expands u is done on-device by the PE instead of on the
host.  Device (per core, batch shard of 128, transposed layout
[dims x batch]): per time step a K=7 Picard fixed-point solve of
  w = tanh(C1' x + D11' w + D12' u)
as PE matmuls accumulating into PSUM + one ACT tanh per iteration, then
x_{t+1} and y_t via two more matmul pairs.  The u-projection matmuls
(D12p/EB2/D22 against the raw u columns) pre-fill each PSUM bank off the
critical chain.  y is returned as bf16 to halve the D2H transfer.

The jitted shard_map callable is cached at module level so repeat calls
skip retracing/lowering (the stock run_bass_kernel_spmd rebuilds it per
call).
"""

import numpy as np

D_IN, D_OUT, D_X, D_NL = 32, 32, 64, 64
EPS, ALPHA = 1e-3, 1.0
N_CORES = 8
BPC = 128          # batch per core
K_ITERS = 7        # Picard tanh rounds (incl. cold-start round)

_BUILD_CACHE = {}
_RUNNER_CACHE = {}


def _bf16_dtype():
    import ml_dtypes
    return ml_dtypes.bfloat16


def _bf16(a):
    return np.asarray(a, dtype=np.float32).astype(_bf16_dtype())


def _derive_mats(X, Y, B2, C2, D21, D22, D12):
    n = 2 * D_X + D_NL
    Xd = np.asarray(X, np.float64)
    Yd = np.asarray(Y, np.float64)
    H = Xd.T @ Xd + EPS * np.eye(n)
    H11 = H[:D_X, :D_X]
    H21 = H[D_X:D_X + D_NL, :D_X]
    H22 = H[D_X:D_X + D_NL, D_X:D_X + D_NL]
    H31 = H[D_X + D_NL:, :D_X]
    H32 = H[D_X + D_NL:, D_X:D_X + D_NL]
    H33 = H[D_X + D_NL:, D_X + D_NL:]
    F_mat, B1 = H31, H32
    E = 0.5 * (H11 + ALPHA * H33 + Yd - Yd.T)
    E_inv = np.linalg.inv(E)
    Lam = 0.5 * np.diag(H22)
    D11 = -np.tril(H22, k=-1)
    C1 = -H21
    iL = (1.0 / Lam)[:, None]
    D11p = (D11 * iL).astype(np.float32)
    C1p = (C1 * iL).astype(np.float32)
    D12p = (np.asarray(D12, np.float64) * iL).astype(np.float32)
    EF = (E_inv @ F_mat).astype(np.float32)
    EB1 = (E_inv @ B1).astype(np.float32)
    EB2 = (E_inv @ np.asarray(B2, np.float64)).astype(np.float32)
    return dict(D11p=D11p, C1p=C1p, D12p=D12p, EF=EF, EB1=EB1, EB2=EB2,
                C2=np.asarray(C2, np.float32), D21=np.asarray(D21, np.float32),
                D22=np.asarray(D22, np.float32))


def _build_program(T):
    """Build the per-core Bass/Tile program (identical for all cores)."""
    from contextlib import ExitStack
    import concourse.bass as bass
    import concourse.tile as tile
    from concourse import bacc, mybir

    bf = mybir.dt.bfloat16
    f32 = mybir.dt.float32
    TANH = mybir.ActivationFunctionType.Tanh

    nc = bacc.Bacc("TRN2", target_bir_lowering=False, debug=False)

    u = nc.dram_tensor("u", [32, T * BPC], bf, kind="ExternalInput")
    wA = nc.dram_tensor("wA", [128, 64], bf, kind="ExternalInput")
    wA0 = nc.dram_tensor("wA0", [128, 64], bf, kind="ExternalInput")
    wWx = nc.dram_tensor("wWx", [128, 64], bf, kind="ExternalInput")
    wWy = nc.dram_tensor("wWy", [128, 32], bf, kind="ExternalInput")
    wU1 = nc.dram_tensor("wU1", [32, 64], bf, kind="ExternalInput")   # D12p^T
    wU2 = nc.dram_tensor("wU2", [32, 64], bf, kind="ExternalInput")   # EB2^T
    wU3 = nc.dram_tensor("wU3", [32, 32], bf, kind="ExternalInput")   # D22^T
    yout = nc.dram_tensor("yout", [32, T * BPC], bf, kind="ExternalOutput")

    with ExitStack() as ctx:
        tc = ctx.enter_context(tile.TileContext(nc))
        const = ctx.enter_context(tc.tile_pool(name="const", bufs=1))

        tA = const.tile([128, 64], bf)
        nc.sync.dma_start(tA[:, :], wA[:, :])
        tA0 = const.tile([128, 64], bf)
        nc.sync.dma_start(tA0[:, :], wA0[:, :])
        tWx = const.tile([128, 64], bf)
        nc.sync.dma_start(tWx[:, :], wWx[:, :])
        tWy = const.tile([128, 32], bf)
        nc.sync.dma_start(tWy[:, :], wWy[:, :])
        tU1 = const.tile([32, 64], bf)
        nc.sync.dma_start(tU1[:, :], wU1[:, :])
        tU2 = const.tile([32, 64], bf)
        nc.sync.dma_start(tU2[:, :], wU2[:, :])
        tU3 = const.tile([32, 32], bf)
        nc.sync.dma_start(tU3[:, :], wU3[:, :])

        tu = const.tile([32, T * BPC], bf)
        NCH = max(1, T // 32)
        CW = T * BPC // NCH
        for i in range(NCH):
            nc.sync.dma_start(tu[:, bass.ts(i, CW)], u[:, bass.ts(i, CW)])

        spool = ctx.enter_context(tc.tile_pool(name="state", bufs=1))
        state = spool.tile([128, BPC], bf)
        # keep every writer of `state` on the ACT engine so downstream
        # instructions never exceed the per-instruction sem-wait limit
        nc.scalar.memzero(state[:, :])

        psw = ctx.enter_context(tc.tile_pool(name="psw", bufs=4, space="PSUM"))
        psx = ctx.enter_context(tc.tile_pool(name="psx", bufs=2, space="PSUM"))
        psy = ctx.enter_context(tc.tile_pool(name="psy", bufs=2, space="PSUM"))
        ystage_pool = ctx.enter_context(tc.tile_pool(name="ystage", bufs=2))

        YCH = min(32, T)  # time steps per output chunk
        for tch in range(T // YCH):
            ystage = ystage_pool.tile([32, YCH * BPC], bf)
            for tt in range(YCH):
                t = tch * YCH + tt
                col = bass.ts(t, BPC)
                for k in range(K_ITERS):
                    # u-projection matmul first (no chain deps): PE pre-fills
                    # the bank while the previous tanh runs, so the
                    # chain-gated wA matmul is the only PE op per link
                    pw = psw.tile([64, BPC], f32)
                    nc.tensor.matmul(pw[:, :], tU1[:, :], tu[:, col],
                                     start=True, stop=False)
                    nc.tensor.matmul(pw[:, :], (tA0 if k == 0 else tA)[:, :],
                                     state[:, :], start=False, stop=True)
                    nc.scalar.activation(state[64:128, :], pw[:, :], TANH)
                px = psx.tile([64, BPC], f32)
                nc.tensor.matmul(px[:, :], tU2[:, :], tu[:, col],
                                 start=True, stop=False)
                nc.tensor.matmul(px[:, :], tWx[:, :], state[:, :],
                                 start=False, stop=True)
                nc.scalar.copy(state[0:64, :], px[:, :])
                py = psy.tile([32, BPC], f32)
                nc.tensor.matmul(py[:, :], tU3[:, :], tu[:, col],
                                 start=True, stop=False)
                nc.tensor.matmul(py[:, :], tWy[:, :], state[:, :],
                                 start=False, stop=True)
                nc.vector.tensor_copy(ystage[:, bass.ts(tt, BPC)], py[:, :])
            nc.sync.dma_start(yout[:, bass.ts(tch, YCH * BPC)], ystage[:, :])

    nc.finalize()
    return nc


def _get_program(T):
    if T not in _BUILD_CACHE:
        _BUILD_CACHE[T] = _build_program(T)
    return _BUILD_CACHE[T]


class _CachedRunner:
    """Jit the shard_map'd bass_exec once and reuse across kernel() calls.

    Mirrors concourse.bass2jax.run_bass_via_pjrt, minus the per-call
    closure rebuild (which forces a retrace + relower every call) and
    minus shipping 32 MB of donated zero output buffers from the host
    (they are created on-device instead).
    """

    def __init__(self, nc):
        import jax
        from jax.sharding import Mesh, PartitionSpec, NamedSharding
        from jax.experimental.shard_map import shard_map
        from concourse import mybir
        from concourse.bass2jax import (_bass_exec_p, install_neuronx_cc_hook,
                                        partition_id_tensor)

        install_neuronx_cc_hook()
        self.jax = jax
        self.nc = nc

        partition_name = (nc.partition_id_tensor.name
                          if nc.partition_id_tensor else None)
        in_names, out_names, out_avals = [], [], []
        for alloc in nc.m.functions[0].allocations:
            if not isinstance(alloc, mybir.MemoryLocationSet):
                continue
            name = alloc.memorylocations[0].name
            if alloc.kind == "ExternalInput":
                if name != partition_name:
                    in_names.append(name)
            elif alloc.kind == "ExternalOutput":
                out_names.append(name)
                shape = tuple(alloc.tensor_shape)
                dtype = mybir.dt.np(alloc.dtype)
                out_avals.append(jax.core.ShapedArray(shape, dtype))
        n_params = len(in_names)
        n_outs = len(out_avals)
        self.in_names = list(in_names)
        self.out_names = list(out_names)
        self.out_avals = out_avals
        all_names = in_names + out_names
        if partition_name is not None:
            all_names.append(partition_name)

        def _body(*args):
            operands = list(args)
            if partition_name is not None:
                operands.append(partition_id_tensor())
            outs = _bass_exec_p.bind(
                *operands,
                out_avals=tuple(out_avals),
                in_names=tuple(all_names),
                out_names=tuple(out_names),
                lowering_input_output_aliases=(),
                sim_require_finite=True,
                sim_require_nnan=True,
                nc=nc,
            )
            return tuple(outs)

        devices = jax.devices()[:N_CORES]
        assert len(devices) == N_CORES
        self.mesh = Mesh(np.asarray(devices), ("core",))
        donate = tuple(range(n_params, n_params + n_outs))
        self.sharded = jax.jit(
            shard_map(_body, mesh=self.mesh,
                      in_specs=(PartitionSpec("core"),) * (n_params + n_outs),
                      out_specs=(PartitionSpec("core"),) * n_outs,
                      check_rep=False),
            donate_argnums=donate, keep_unused=True)

        sh = NamedSharding(self.mesh, PartitionSpec("core"))
        import jax.numpy as jnp

        def _mk_zeros():
            return tuple(
                jnp.zeros((N_CORES * a.shape[0],) + tuple(a.shape[1:]), a.dtype)
                for a in out_avals)

        self.make_zeros = jax.jit(_mk_zeros, out_shardings=(sh,) * n_outs)

    def __call__(self, concat_inputs):
        """concat_inputs: dict name -> global np/jax array (cores stacked on
        axis 0). Returns list of np arrays (one per output), global shape."""
        args = [concat_inputs[n] for n in self.in_names]
        zeros = self.make_zeros()
        outs = self.sharded(*args, *zeros)
        return [np.asarray(o) for o in outs]


def _get_runner(T):
    if T not in _RUNNER_CACHE:
        _RUNNER_CACHE[T] = _CachedRunner(_get_program(T))
    return _RUNNER_CACHE[T]


def kernel(u_in, X, Y, B2, C2, D21, D22, D12):
    u_in = np.asarray(u_in, np.float32)
    B, T, _ = u_in.shape
    assert B == N_CORES * BPC

    m = _derive_mats(X, Y, B2, C2, D21, D22, D12)

    wA = np.vstack([m["C1p"].T, m["D11p"].T])            # [128, 64]
    wA0 = np.vstack([m["C1p"].T, np.zeros((64, 64), np.float32)])
    wWx = np.vstack([m["EF"].T, m["EB1"].T])             # [128, 64]
    wWy = np.vstack([m["C2"].T, m["D21"].T])             # [128, 32]
    wU1 = m["D12p"].T.copy()                             # [32, 64]
    wU2 = m["EB2"].T.copy()                              # [32, 64]
    wU3 = m["D22"].T.copy()                              # [32, 32]

    runner = _get_runner(T)

    # [8*32, T*128] bf16: per-core slice c is u_t^T laid out time-major
    u_cat = _bf16(np.ascontiguousarray(
        u_in.reshape(N_CORES, BPC, T, D_IN).transpose(0, 3, 2, 1)
    ).reshape(N_CORES * D_IN, T * BPC))

    def rep(a):
        return _bf16(np.concatenate([a] * N_CORES, axis=0))

    concat_inputs = {
        "u": u_cat,
        "wA": rep(wA), "wA0": rep(wA0), "wWx": rep(wWx), "wWy": rep(wWy),
        "wU1": rep(wU1), "wU2": rep(wU2), "wU3": rep(wU3),
    }
    outs = runner(concat_inputs)
    y = outs[runner.out_names.index("yout")]             # [8*32, T*128] bf16

    out = np.ascontiguousarray(
        y.reshape(N_CORES, D_OUT, T, BPC).transpose(0, 3, 2, 1)
        .astype(np.float32)
    ).reshape(B, T, D_OUT)
    return out


# revision 4
# speedup vs baseline: 1.2227x; 1.2227x over previous
"""ContractiveREN Trainium2 kernel.

Host: derive REN model matrices from (X, Y, ...) in numpy (pre-scale by
1/Lambda, fold E_inv into the state-update matrices), transpose u to
[dims x time*batch] bf16, and ship ONLY the raw 32-row u to each core --
the axon tunnel (~45 MB/s, full duplex) is the bottleneck, so every
projection that expands u is done on-device by the PE instead of on the
host.  Device (per core, transposed layout [dims x batch]): per time
step a K=7 Picard fixed-point solve of
  w = tanh(C1' x + D11' w + D12' u)
as PE matmuls accumulating into PSUM + one ACT tanh per iteration, then
x_{t+1} and y_t via two more matmul pairs.  The u-projection matmuls
(D12p/EB2/D22 against the raw u columns) pre-fill each PSUM bank off the
critical chain.  y is returned as bf16 to halve the D2H transfer.

Each core's batch of 128 is split into 4 independent chunks of 32 that
run as separate SPMD calls: chunk j+1's upload overlaps chunk j's
execution and download (the tunnel is full duplex), and host-side
dtype conversion overlaps the transfers.  Weights are device_put once
per call and reused by all chunks.  The jitted shard_map callable is
cached at module level so repeat calls skip retracing/lowering.
"""

import numpy as np

D_IN, D_OUT, D_X, D_NL = 32, 32, 64, 64
EPS, ALPHA = 1e-3, 1.0
N_CORES = 8
BPC = 128          # batch per core
N_CHUNK = 4        # independent batch chunks per core (pipelined calls)
K_ITERS = 7        # Picard tanh rounds (incl. cold-start round)

_BUILD_CACHE = {}
_RUNNER_CACHE = {}
_BF16 = None


def _bf16_dtype():
    global _BF16
    if _BF16 is None:
        import ml_dtypes
        _BF16 = ml_dtypes.bfloat16
    return _BF16


def _bf16(a):
    return np.asarray(a, dtype=np.float32).astype(_bf16_dtype())


def _derive_mats(X, Y, B2, C2, D21, D22, D12):
    n = 2 * D_X + D_NL
    Xd = np.asarray(X, np.float64)
    Yd = np.asarray(Y, np.float64)
    H = Xd.T @ Xd + EPS * np.eye(n)
    H11 = H[:D_X, :D_X]
    H21 = H[D_X:D_X + D_NL, :D_X]
    H22 = H[D_X:D_X + D_NL, D_X:D_X + D_NL]
    H31 = H[D_X + D_NL:, :D_X]
    H32 = H[D_X + D_NL:, D_X:D_X + D_NL]
    H33 = H[D_X + D_NL:, D_X + D_NL:]
    F_mat, B1 = H31, H32
    E = 0.5 * (H11 + ALPHA * H33 + Yd - Yd.T)
    E_inv = np.linalg.inv(E)
    Lam = 0.5 * np.diag(H22)
    D11 = -np.tril(H22, k=-1)
    C1 = -H21
    iL = (1.0 / Lam)[:, None]
    D11p = (D11 * iL).astype(np.float32)
    C1p = (C1 * iL).astype(np.float32)
    D12p = (np.asarray(D12, np.float64) * iL).astype(np.float32)
    EF = (E_inv @ F_mat).astype(np.float32)
    EB1 = (E_inv @ B1).astype(np.float32)
    EB2 = (E_inv @ np.asarray(B2, np.float64)).astype(np.float32)
    return dict(D11p=D11p, C1p=C1p, D12p=D12p, EF=EF, EB1=EB1, EB2=EB2,
                C2=np.asarray(C2, np.float32), D21=np.asarray(D21, np.float32),
                D22=np.asarray(D22, np.float32))


def _build_program(T, bpc):
    """Build the per-core Bass/Tile program (identical for all cores)."""
    from contextlib import ExitStack
    import concourse.bass as bass
    import concourse.tile as tile
    from concourse import bacc, mybir

    bf = mybir.dt.bfloat16
    f32 = mybir.dt.float32
    TANH = mybir.ActivationFunctionType.Tanh

    nc = bacc.Bacc("TRN2", target_bir_lowering=False, debug=False)

    u = nc.dram_tensor("u", [32, T * bpc], bf, kind="ExternalInput")
    wA = nc.dram_tensor("wA", [128, 64], bf, kind="ExternalInput")
    wA0 = nc.dram_tensor("wA0", [128, 64], bf, kind="ExternalInput")
    wWx = nc.dram_tensor("wWx", [128, 64], bf, kind="ExternalInput")
    wWy = nc.dram_tensor("wWy", [128, 32], bf, kind="ExternalInput")
    wU1 = nc.dram_tensor("wU1", [32, 64], bf, kind="ExternalInput")   # D12p^T
    wU2 = nc.dram_tensor("wU2", [32, 64], bf, kind="ExternalInput")   # EB2^T
    wU3 = nc.dram_tensor("wU3", [32, 32], bf, kind="ExternalInput")   # D22^T
    yout = nc.dram_tensor("yout", [32, T * bpc], bf, kind="ExternalOutput")

    with ExitStack() as ctx:
        tc = ctx.enter_context(tile.TileContext(nc))
        const = ctx.enter_context(tc.tile_pool(name="const", bufs=1))

        tA = const.tile([128, 64], bf)
        nc.sync.dma_start(tA[:, :], wA[:, :])
        tA0 = const.tile([128, 64], bf)
        nc.sync.dma_start(tA0[:, :], wA0[:, :])
        tWx = const.tile([128, 64], bf)
        nc.sync.dma_start(tWx[:, :], wWx[:, :])
        tWy = const.tile([128, 32], bf)
        nc.sync.dma_start(tWy[:, :], wWy[:, :])
        tU1 = const.tile([32, 64], bf)
        nc.sync.dma_start(tU1[:, :], wU1[:, :])
        tU2 = const.tile([32, 64], bf)
        nc.sync.dma_start(tU2[:, :], wU2[:, :])
        tU3 = const.tile([32, 32], bf)
        nc.sync.dma_start(tU3[:, :], wU3[:, :])

        tu = const.tile([32, T * bpc], bf)
        NCH = max(1, T // 32)
        CW = T * bpc // NCH
        for i in range(NCH):
            nc.sync.dma_start(tu[:, bass.ts(i, CW)], u[:, bass.ts(i, CW)])

        spool = ctx.enter_context(tc.tile_pool(name="state", bufs=1))
        state = spool.tile([128, bpc], bf)
        # keep every writer of `state` on the ACT engine so downstream
        # instructions never exceed the per-instruction sem-wait limit
        nc.scalar.memzero(state[:, :])

        psw = ctx.enter_context(tc.tile_pool(name="psw", bufs=4, space="PSUM"))
        psx = ctx.enter_context(tc.tile_pool(name="psx", bufs=2, space="PSUM"))
        psy = ctx.enter_context(tc.tile_pool(name="psy", bufs=2, space="PSUM"))
        ystage_pool = ctx.enter_context(tc.tile_pool(name="ystage", bufs=2))

        YCH = min(32, T)  # time steps per output chunk
        for tch in range(T // YCH):
            ystage = ystage_pool.tile([32, YCH * bpc], bf)
            for tt in range(YCH):
                t = tch * YCH + tt
                col = bass.ts(t, bpc)
                for k in range(K_ITERS):
                    # u-projection matmul first (no chain deps): PE pre-fills
                    # the bank while the previous tanh runs, so the
                    # chain-gated wA matmul is the only PE op per link
                    pw = psw.tile([64, bpc], f32)
                    nc.tensor.matmul(pw[:, :], tU1[:, :], tu[:, col],
                                     start=True, stop=False)
                    nc.tensor.matmul(pw[:, :], (tA0 if k == 0 else tA)[:, :],
                                     state[:, :], start=False, stop=True)
                    nc.scalar.activation(state[64:128, :], pw[:, :], TANH)
                px = psx.tile([64, bpc], f32)
                nc.tensor.matmul(px[:, :], tU2[:, :], tu[:, col],
                                 start=True, stop=False)
                nc.tensor.matmul(px[:, :], tWx[:, :], state[:, :],
                                 start=False, stop=True)
                nc.scalar.copy(state[0:64, :], px[:, :])
                py = psy.tile([32, bpc], f32)
                nc.tensor.matmul(py[:, :], tU3[:, :], tu[:, col],
                                 start=True, stop=False)
                nc.tensor.matmul(py[:, :], tWy[:, :], state[:, :],
                                 start=False, stop=True)
                nc.vector.tensor_copy(ystage[:, bass.ts(tt, bpc)], py[:, :])
            nc.sync.dma_start(yout[:, bass.ts(tch, YCH * bpc)], ystage[:, :])

    nc.finalize()
    return nc


def _get_program(key):
    if key not in _BUILD_CACHE:
        T, bpc = key if isinstance(key, tuple) else (key, BPC)
        _BUILD_CACHE[key] = _build_program(T, bpc)
    return _BUILD_CACHE[key]


class _CachedRunner:
    """Jit the shard_map'd bass_exec once and reuse across kernel() calls.

    Mirrors concourse.bass2jax.run_bass_via_pjrt, minus the per-call
    closure rebuild (which forces a retrace + relower every call) and
    minus shipping donated zero output buffers from the host (they are
    created on-device instead).
    """

    def __init__(self, nc):
        import jax
        from jax.sharding import Mesh, PartitionSpec, NamedSharding
        from jax.experimental.shard_map import shard_map
        from concourse import mybir
        from concourse.bass2jax import (_bass_exec_p, install_neuronx_cc_hook,
                                        partition_id_tensor)

        install_neuronx_cc_hook()
        self.jax = jax
        self.nc = nc

        partition_name = (nc.partition_id_tensor.name
                          if nc.partition_id_tensor else None)
        in_names, out_names, out_avals = [], [], []
        for alloc in nc.m.functions[0].allocations:
            if not isinstance(alloc, mybir.MemoryLocationSet):
                continue
            name = alloc.memorylocations[0].name
            if alloc.kind == "ExternalInput":
                if name != partition_name:
                    in_names.append(name)
            elif alloc.kind == "ExternalOutput":
                out_names.append(name)
                shape = tuple(alloc.tensor_shape)
                dtype = mybir.dt.np(alloc.dtype)
                out_avals.append(jax.core.ShapedArray(shape, dtype))
        n_params = len(in_names)
        n_outs = len(out_avals)
        self.in_names = list(in_names)
        self.out_names = list(out_names)
        self.out_avals = out_avals
        all_names = in_names + out_names
        if partition_name is not None:
            all_names.append(partition_name)

        def _body(*args):
            operands = list(args)
            if partition_name is not None:
                operands.append(partition_id_tensor())
            outs = _bass_exec_p.bind(
                *operands,
                out_avals=tuple(out_avals),
                in_names=tuple(all_names),
                out_names=tuple(out_names),
                lowering_input_output_aliases=(),
                sim_require_finite=True,
                sim_require_nnan=True,
                nc=nc,
            )
            return tuple(outs)

        devices = jax.devices()[:N_CORES]
        assert len(devices) == N_CORES
        self.mesh = Mesh(np.asarray(devices), ("core",))
        self.sh = NamedSharding(self.mesh, PartitionSpec("core"))
        donate = tuple(range(n_params, n_params + n_outs))
        self.sharded = jax.jit(
            shard_map(_body, mesh=self.mesh,
                      in_specs=(PartitionSpec("core"),) * (n_params + n_outs),
                      out_specs=(PartitionSpec("core"),) * n_outs,
                      check_rep=False),
            donate_argnums=donate, keep_unused=True)

        import jax.numpy as jnp

        def _mk_zeros():
            return tuple(
                jnp.zeros((N_CORES * a.shape[0],) + tuple(a.shape[1:]), a.dtype)
                for a in out_avals)

        self.make_zeros = jax.jit(_mk_zeros, out_shardings=(self.sh,) * n_outs)


def _get_runner(key):
    if key not in _RUNNER_CACHE:
        _RUNNER_CACHE[key] = _CachedRunner(_get_program(key))
    return _RUNNER_CACHE[key]


def kernel(u_in, X, Y, B2, C2, D21, D22, D12):
    import jax
    bf = _bf16_dtype()
    u_in = np.ascontiguousarray(np.asarray(u_in, np.float32))
    B, T, _ = u_in.shape
    assert B == N_CORES * BPC
    bpc = BPC // N_CHUNK

    m = _derive_mats(X, Y, B2, C2, D21, D22, D12)
    runner = _get_runner((T, bpc))

    def rep(a):
        return _bf16(np.concatenate([a] * N_CORES, axis=0))

    # ship weights once (async), reused by every chunk call
    wdev = {
        "wA": rep(np.vstack([m["C1p"].T, m["D11p"].T])),
        "wA0": rep(np.vstack([m["C1p"].T, np.zeros((64, 64), np.float32)])),
        "wWx": rep(np.vstack([m["EF"].T, m["EB1"].T])),
        "wWy": rep(np.vstack([m["C2"].T, m["D21"].T])),
        "wU1": rep(m["D12p"].T.copy()),
        "wU2": rep(m["EB2"].T.copy()),
        "wU3": rep(m["D22"].T.copy()),
    }
    wdev = {k: jax.device_put(v, runner.sh) for k, v in wdev.items()}

    yi = runner.out_names.index("yout")
    u5 = u_in.reshape(N_CORES, N_CHUNK, bpc, T, D_IN)
    futs = []
    for j in range(N_CHUNK):
        # [core, dim, T, bpc] -> bf16 contiguous -> [8*32, T*bpc]
        ucat = (u5[:, j].transpose(0, 3, 2, 1)
                .astype(bf, order="C")
                .reshape(N_CORES * D_IN, T * bpc))
        udev = jax.device_put(ucat, runner.sh)  # async upload starts now
        zeros = runner.make_zeros()
        args = [udev if n == "u" else wdev[n] for n in runner.in_names]
        outs = runner.sharded(*args, *zeros)
        futs.append(outs)
        try:
            outs[yi].copy_to_host_async()
        except Exception:
            pass

    out = np.empty((B, T, D_OUT), np.float32)
    out5 = out.reshape(N_CORES, N_CHUNK, bpc, T, D_OUT)
    for j, outs in enumerate(futs):
        yj = np.asarray(outs[yi])                      # [8*32, T*bpc] bf16
        out5[:, j] = (yj.reshape(N_CORES, D_OUT, T, bpc)
                      .transpose(0, 3, 2, 1))
    return out
